# revision 18
# baseline (speedup 1.0000x reference)
"""Bass/Trainium2 kernel for nn_BiasedCrossAttention (B=2, Lq=Lk=1024, D=1024, H=16).

Sharding: 8 cores = 2 batches x 4 head-groups (4 heads each, tensor-parallel).

Structure (v2 — rebuilt from trace analysis of the 81us baseline):
  - Host-side key compaction: masked keys are dropped exactly (softmax prob 0),
    K/V/bias gathered to the unmasked keys and padded to LKP=640 columns with
    bias=-30000 so pads contribute exactly nothing.
  - Inputs pre-tiled host-side to [128, t, l] contiguous; q/k split into
    first-use-sized chunks and DMA'd in consumption order so the first real
    matmul starts as soon as wq+q-half land (~13us) instead of after the
    full preamble.
  - PE warm/ramp dummies depend only on GpSimd-memset tiles (not on any DMA),
    so they fill the DMA preamble instead of serializing after it.
  - Softmax: scores stay in PSUM; GpSimd drains them with a fused
    (s*0.125 + bias) scalar_tensor_tensor into f32 SBUF (psum banks recycle at
    GpSimd speed, and the eb-dependence lives on the GpSimd queue only, so
    PE-side projection drains on DVE can never head-of-line block on eb);
    ScalarE then does a single Exp per tile straight to bf16.  bias is shipped
    raw in fp16 (no host-side exp), masked/padded keys get -30000.
  - Softmax denominator via the ones-columns trick in vh (free: PV matmul
    output partitions 64..127 replicate the denominator), reciprocal via the
    single-op DVE reciprocal_approx_fast instead of a ScalarE Ln/Exp chain.
  - bv and bo are folded host-side into a single output bias
    (out += (attn + bv) @ Wo.T + bo == attn @ Wo.T + (Wo@bv + bo)), dropping
    the vproj ones-row (VDT 9 -> 8).
  - o_proj psum->bf16 casts split GpSimd (lt=0) / DVE (lt=1) so the final
    cast+DMA tail is not queued behind other work.

Per core (batch b, head-group g):
  - qhT/khT [m=256, L] projections (m = group's head dims), bias folded into
    the psum->sbuf DVE copy
  - vh [LKP, per-head 64 v-cols + 64 ones-cols]
  - probs = exp(scores/8 + bias), no max-subtraction (safe at these magnitudes)
  - o_proj partial outT [D, Lq] in bf16, summed over the 4 groups host-side.
"""

import numpy as np
import ml_dtypes

import bass_rust
import concourse.bass as bass
import concourse.tile as tile
import concourse.mybir as mybir
from concourse.bass_utils import run_bass_kernel_spmd
from concourse.vector_clock import ScopedClock

F32 = mybir.dt.float32
F16 = mybir.dt.float16
BF16 = mybir.dt.bfloat16
AT = mybir.ActivationFunctionType
ALU = mybir.AluOpType

B, LQ, LK, D, H = 2, 1024, 1024, 1024, 16
DH = D // H              # 64
GROUPS = 4               # head-groups across cores (x B batches = 8 cores)
GH = H // GROUPS         # heads per group
M = GH * DH              # 256 projected dims per group
NCORES = 8
P = 128
DT = D // P              # 8 d-tiles
LKP = 640                # padded compacted key count
KT = LKP // P            # 5 lk-tiles
KS0 = 512                # first k-chunk (matches kproj_a)
LQT = 512                # lq tile (psum free limit)
NLT = LQ // LQT          # 2 lq-tiles
SCALE = 1.0 / np.sqrt(DH)
NEGB = -30000.0          # bias for masked/padded keys: exp underflows to 0
NDUMMY = 16              # PE warm/ramp matmuls covering the DMA preamble
DUMW = 384               # dummy matmul free size

CDT = BF16               # matmul compute dtype
NP_CDT = ml_dtypes.bfloat16

_counter = [0]


def _split_waits_in_list(nc, insts):
    """This walrus build rejects >1 embedded sync-wait per instruction; move
    extra waits onto standalone EventSemaphore instructions just before."""
    out = []
    for ins in insts:
        si = getattr(ins, "sync_info", None)
        if si is not None and len(si.on_wait) > 1:
            extra = list(si.on_wait[:-1])
            del si.on_wait[:-1]
            for w in extra:
                _counter[0] += 1
                ev = bass_rust.InstEventSemaphore(
                    name=f"I-xw{_counter[0]}", ins=[], outs=[])
                ev.engine = ins.engine
                ev.sync_info = mybir.SyncInfo(on_wait=[w], on_update=[])
                try:
                    ev.debug = ins.debug
                except Exception:
                    pass
                nc.register_instruction(ev)
                out.append(ev)
        out.append(ins)
    return out


class PatchedTileContext(tile.TileContext):
    def _lower_ordered_insts(self, ordered):
        for name in list(ordered.keys()):
            ordered[name] = _split_waits_in_list(self.nc, ordered[name])
        return super()._lower_ordered_insts(ordered)

    def _drain_and_barrier(self, tick_clock, wait_clock):
        nc = self.nc
        drain_inst = nc.sync.drain()
        wait_clock.add_sem_waits(
            drain_inst.ins, ScopedClock({None: tick_clock.global_clock}))
        si = drain_inst.ins.sync_info
        waits = list(si.on_wait)
        if len(waits) > 1:
            del si.on_wait[1:]
            for w in waits[1:]:
                nop = nc.sync.nop(nofuse=True)
                if nop.ins.sync_info is None:
                    nop.ins.sync_info = mybir.SyncInfo(on_wait=[], on_update=[])
                nop.ins.sync_info.on_wait.append(w)
        nc.all_engine_barrier()
        assert self.sems is not None
        popped = nc._tile_sem_poison_stack.pop()
        assert popped is self._sem_poison
        nc.clear_and_free_semaphores(list(self.sems.allocated().values()))
        nc.all_engine_barrier()


def build_program():
    nc = bass.Bass()

    HE = 2 * DH

    # inputs are host-pre-tiled to [128, n*l] contiguous (partition-major)
    qT0 = nc.dram_tensor("qT0", [P, DT * LQT], CDT, kind="ExternalInput")
    qT1 = nc.dram_tensor("qT1", [P, DT * LQT], CDT, kind="ExternalInput")
    kT0 = nc.dram_tensor("kT0", [P, DT * KS0], CDT, kind="ExternalInput")
    kT1 = nc.dram_tensor("kT1", [P, DT * (LKP - KS0)], CDT, kind="ExternalInput")
    vT = nc.dram_tensor("vT", [P, DT * LKP], CDT, kind="ExternalInput")
    wqT = nc.dram_tensor("wqT", [P, DT * M], CDT, kind="ExternalInput")
    wkT = nc.dram_tensor("wkT", [P, DT * M], CDT, kind="ExternalInput")
    wvT = nc.dram_tensor("wvT", [P, DT * M], CDT, kind="ExternalInput")
    woT = nc.dram_tensor("woT", [P, (M // P) * D], CDT, kind="ExternalInput")
    ebT = nc.dram_tensor("ebT", [P, KT * LQ], CDT, kind="ExternalInput")
    bqk = nc.dram_tensor("bqk", [P, 4], F32, kind="ExternalInput")
    outT = nc.dram_tensor("outT", [D, LQ], CDT, kind="ExternalOutput")

    from contextlib import ExitStack
    with PatchedTileContext(nc) as tc, ExitStack() as ctx:
        consts = ctx.enter_context(tc.tile_pool(name="consts", bufs=1))
        tmp = ctx.enter_context(tc.tile_pool(name="tmp", bufs=8))
        rcp = ctx.enter_context(tc.tile_pool(name="rcp", bufs=4))
        exps = ctx.enter_context(tc.tile_pool(name="exps", bufs=15))
        outp = ctx.enter_context(tc.tile_pool(name="outp", bufs=4))
        ps_pp = ctx.enter_context(tc.tile_pool(name="ps_pp", bufs=2, space="PSUM"))
        ps_sc = ctx.enter_context(tc.tile_pool(name="ps_sc", bufs=2, space="PSUM"))
        ps_pv = ctx.enter_context(tc.tile_pool(name="ps_pv", bufs=2, space="PSUM"))

        def flat(t3):
            return t3.rearrange("p t l -> p (t l)")

        # ---- input loads: consumption-ordered, contiguous ----------------
        # v is shipped k-tile-major so vproj(k) can start per 0.26MB chunk.
        bq_t = consts.tile([P, 4], F32, name="bq_t")
        nc.sync.dma_start(bq_t[:], bqk[:])
        wq_a = consts.tile([P, DT, M], CDT, name="wq_a")
        nc.sync.dma_start(flat(wq_a[:]), wqT[:])
        q_a = [consts.tile([P, DT, LQT], CDT, name=f"q_a{t}") for t in range(2)]
        nc.sync.dma_start(flat(q_a[0][:]), qT0[:])
        wk_a = consts.tile([P, DT, M], CDT, name="wk_a")
        nc.sync.dma_start(flat(wk_a[:]), wkT[:])
        k_a0 = consts.tile([P, DT, KS0], CDT, name="k_a0")
        nc.sync.dma_start(flat(k_a0[:]), kT0[:])
        eb_a = [consts.tile([P, LQ], CDT, name=f"eb_a{k}") for k in range(KT)]
        nc.sync.dma_start(eb_a[0][:], ebT[:, 0 * LQ:1 * LQ])
        nc.sync.dma_start(flat(q_a[1][:]), qT1[:])
        nc.sync.dma_start(eb_a[1][:], ebT[:, 1 * LQ:2 * LQ])
        k_a1 = consts.tile([P, DT, LKP - KS0], CDT, name="k_a1")
        nc.sync.dma_start(flat(k_a1[:]), kT1[:])
        wv_a = consts.tile([P, DT, M], CDT, name="wv_a")
        nc.sync.dma_start(flat(wv_a[:]), wvT[:])
        v_a = consts.tile([P, KT, DT, P], CDT, name="v_a")
        vf = v_a[:].rearrange("p k t c -> p (k t c)")
        KCH = DT * P
        nc.sync.dma_start(vf[:, 0 * KCH:1 * KCH], vT[:, 0 * KCH:1 * KCH])
        nc.sync.dma_start(vf[:, 1 * KCH:2 * KCH], vT[:, 1 * KCH:2 * KCH])
        nc.sync.dma_start(eb_a[2][:], ebT[:, 2 * LQ:3 * LQ])
        nc.sync.dma_start(vf[:, 2 * KCH:3 * KCH], vT[:, 2 * KCH:3 * KCH])
        nc.sync.dma_start(eb_a[3][:], ebT[:, 3 * LQ:4 * LQ])
        nc.sync.dma_start(vf[:, 3 * KCH:4 * KCH], vT[:, 3 * KCH:4 * KCH])
        nc.sync.dma_start(eb_a[4][:], ebT[:, 4 * LQ:5 * LQ])
        nc.sync.dma_start(vf[:, 4 * KCH:5 * KCH], vT[:, 4 * KCH:5 * KCH])
        wo_a = consts.tile([P, M // P, D], CDT, name="wo_a")
        nc.sync.dma_start(flat(wo_a[:]), woT[:])

        # ---- warmers ------------------------------------------------------
        # ones/vh memsets on GpSimd, exp table load on Scalar, then a run of
        # DMA-independent dummy matmuls that keep the PE busy (and its p-state
        # ramped) through the DMA preamble.
        ones0 = consts.tile([P, P], CDT, name="ones0")
        nc.gpsimd.memset(ones0[:], 1.0)
        ones1 = consts.tile([P, DUMW], CDT, name="ones1")
        nc.gpsimd.memset(ones1[:], 1.0)
        vh = [consts.tile([P, GH * HE], CDT, name=f"vh{k}") for k in range(KT)]
        for k in range(KT):
            nc.gpsimd.memset(vh[k][:], 1.0)
        wsrc = consts.tile([P, 8], F32, name="wsrc")
        nc.gpsimd.memset(wsrc[:], 0.0)
        wdst = tmp.tile([P, 8], F32, name="wdst")
        nc.scalar.activation(wdst[:], wsrc[:], AT.Exp)
        for w in range(NDUMMY):
            dmy = ps_pp.tile([P, DUMW], F32, name="dmy", tag="pp")
            nc.tensor.matmul(dmy[:], ones0[:], ones1[:], start=True, stop=True)

        # ---- projections --------------------------------------------------
        qh = [consts.tile([P, LQ], CDT, name=f"qh{p}") for p in range(M // P)]
        kh = [consts.tile([P, LKP], CDT, name=f"kh{p}") for p in range(M // P)]

        def qproj(lt):
            for p in range(M // P):
                pq = ps_pp.tile([P, LQT], F32, name="pq", tag="pp")
                for i in range(DT):
                    nc.tensor.matmul(pq[:], wq_a[:, i, P * p:P * (p + 1)],
                                     q_a[lt][:, i, :],
                                     start=(i == 0), stop=(i == DT - 1))
                nc.vector.tensor_scalar_add(qh[p][:, LQT * lt:LQT * (lt + 1)],
                                            pq[:], bq_t[:, 2 * p:2 * p + 1])

        def kproj(src, c0, c1):
            for p in range(M // P):
                pk = ps_pp.tile([P, c1 - c0], F32, name="pk", tag="pp")
                for i in range(DT):
                    nc.tensor.matmul(pk[:], wk_a[:, i, P * p:P * (p + 1)],
                                     src[:, i, :],
                                     start=(i == 0), stop=(i == DT - 1))
                nc.vector.tensor_scalar_add(kh[p][:, c0:c1], pk[:],
                                            bq_t[:, 2 * p + 1:2 * p + 2])

        def vproj(k):
            # vh [LKP, GH*128]: per head 64 v-cols + 64 ones-cols; the PV
            # matmul then replicates the denominator across partitions 64..127
            pv = ps_pp.tile([P, M], F32, name="pvproj", tag="pp")
            for i in range(DT):
                nc.tensor.matmul(pv[:], v_a[:, k, i, :], wv_a[:, i, :],
                                 start=(i == 0), stop=(i == DT - 1))
            nc.vector.tensor_copy(
                vh[k][:, :].rearrange("p (h e) -> p h e", e=HE)[:, :, 0:DH],
                pv[:].rearrange("p (h e) -> p h e", e=DH))

        # ---- attention + output projection, lq-tile major -----------------
        attnT = [consts.tile([P, LQ], CDT, name=f"attnT{p}") for p in range(M // P)]

        def sc_tile(lt, hp, k, exf_t):
            # scores for one k-tile -> psum [128,(hl,512)]; ScalarE drains it
            # with a single Exp (psum recycle is eb-independent), GpSimd then
            # multiplies in exp(bias) (SBUF-only; GpSimd cannot touch PSUM).
            lq = slice(LQT * lt, LQT * (lt + 1))
            sps = ps_sc.tile([P, LQ], F32, name="sps", tag="sc")
            for hl in range(2):
                nc.tensor.matmul(sps[:, LQT * hl:LQT * (hl + 1)],
                                 kh[hp][DH * hl:DH * (hl + 1), P * k:P * (k + 1)],
                                 qh[hp][DH * hl:DH * (hl + 1), lq],
                                 start=True, stop=True)
            exf = tmp.tile([P, LQ], CDT, name="exf")
            nc.scalar.activation(exf[:], sps[:], AT.Exp, scale=float(SCALE))
            exf_t.append(exf)

        def sc_muls(lt, hp, exf_t, ex_t, kr=None):
            # one DVE op per k-tile: eb broadcasts over the two hl halves via
            # a stride-0 AP dim (GpSimd is ~2.8x slower per op - keep it out)
            lq = slice(LQT * lt, LQT * (lt + 1))
            for k in (range(KT) if kr is None else kr):
                ex = exps.tile([P, LQ], CDT, name="ex")
                ebs3 = eb_a[k][:, lq].rearrange("p (o l) -> p o l", o=1)
                exf3 = exf_t[k][:].rearrange("p (h l) -> p h l", h=2)
                exf3b, ebs3b = bass.broadcast_tensor_aps(exf3, ebs3)
                nc.vector.tensor_mul(ex[:].rearrange("p (h l) -> p h l", h=2),
                                     exf3b, ebs3b)
                ex_t.append(ex)

        def pv_mm(lt, hp, hl, ex_t):
            h = 2 * hp + hl
            pvo = ps_pv.tile([P, LQT], F32, name="pvo", tag="pv")
            for k in range(KT):
                nc.tensor.matmul(pvo[:], vh[k][:, HE * h:HE * (h + 1)],
                                 ex_t[k][:, LQT * hl:LQT * (hl + 1)],
                                 start=(k == 0), stop=(k == KT - 1))
            return pvo

        def pv_drain(lt, hp, pvos):
            # attn = pv * (1/denominator); denominator replicated in 64:128.
            # Both hl denominators pack into one [128,512] SBUF tile so a
            # single ScalarE Ln+Exp pair serves the whole (lt,hp) block,
            # keeping the Scalar queue mostly free for the exp stream.
            lq = slice(LQT * lt, LQT * (lt + 1))
            den = rcp.tile([P, LQT], F32, name="den")
            for hl in range(2):
                nc.vector.tensor_copy(den[DH * hl:DH * (hl + 1), :],
                                      pvos[hl][DH:P, :])
            lnd = rcp.tile([P, LQT], F32, name="lnd")
            nc.scalar.activation(lnd[:], den[:], AT.Ln)
            rec = rcp.tile([P, LQT], F32, name="rec")
            nc.scalar.activation(rec[:], lnd[:], AT.Exp, scale=-1.0)
            for hl in range(2):
                nc.vector.tensor_mul(attnT[hp][DH * hl:DH * (hl + 1), lq],
                                     pvos[hl][0:DH, :],
                                     rec[DH * hl:DH * (hl + 1), :])

        def oproj(lt, cast_eng, ots=None):
            lq = slice(LQT * lt, LQT * (lt + 1))
            for ot in (range(DT) if ots is None else ots):
                po = ps_pp.tile([P, LQT], F32, name="po", tag="pp")
                for p in range(M // P):
                    nc.tensor.matmul(po[:], wo_a[:, p, P * ot:P * (ot + 1)],
                                     attnT[p][:, lq],
                                     start=(p == 0), stop=(p == M // P - 1))
                osb = outp.tile([P, LQT], CDT, name="osb")
                if cast_eng is nc.scalar:
                    nc.scalar.copy(osb[:], po[:])
                else:
                    cast_eng.tensor_copy(osb[:], po[:])
                nc.sync.dma_start(outT[P * ot:P * (ot + 1), lq], osb[:])

        # ---- schedule -----------------------------------------------------
        # Two pacing facts drive this order: (1) the scores psum pool recycles
        # at ScalarE exp speed (~1us/tile vs ~0.45us PE/tile), so score tiles
        # are interleaved with dense fill (projections, vproj chunks, pv
        # blocks, oproj chunks); (2) the PE drops to its mid p-state after
        # idle/sparse stretches, so the fill also keeps the clock ramped.
        # pv drains are emitted right after their pv pair so the 2-slot pv
        # psum pool unblocks quickly.
        exf = [[] for _ in range(4)]   # raw exp(scores/8) per (lt,hp)
        ex = [[] for _ in range(4)]    # exp * exp(bias), PV input
        g00, g01, g10, g11 = 0, 1, 2, 3

        qproj(0)
        kproj(k_a0, 0, KS0)
        sc_tile(0, 0, 0, exf[g00])
        sc_tile(0, 0, 1, exf[g00])
        qproj(1)
        sc_tile(0, 0, 2, exf[g00])
        sc_tile(0, 0, 3, exf[g00])
        kproj(k_a1, KS0, LKP)
        sc_tile(0, 1, 0, exf[g01])
        sc_tile(0, 1, 1, exf[g01])
        vproj(0)
        sc_tile(0, 1, 2, exf[g01])
        vproj(1)
        sc_tile(0, 1, 3, exf[g01])
        vproj(2)
        sc_tile(0, 0, 4, exf[g00])
        sc_muls(0, 0, exf[g00], ex[g00])
        vproj(3)
        sc_tile(0, 1, 4, exf[g01])
        sc_muls(0, 1, exf[g01], ex[g01])
        vproj(4)
        sc_tile(1, 0, 0, exf[g10])
        sc_tile(1, 0, 1, exf[g10])
        pv00a = pv_mm(0, 0, 0, ex[g00])
        sc_tile(1, 0, 2, exf[g10])
        pv00b = pv_mm(0, 0, 1, ex[g00])
        pv_drain(0, 0, (pv00a, pv00b))
        sc_tile(1, 0, 3, exf[g10])
        pv01a = pv_mm(0, 1, 0, ex[g01])
        sc_tile(1, 0, 4, exf[g10])
        sc_muls(1, 0, exf[g10], ex[g10])
        pv01b = pv_mm(0, 1, 1, ex[g01])
        pv_drain(0, 1, (pv01a, pv01b))
        sc_tile(1, 1, 0, exf[g11])
        pv10a = pv_mm(1, 0, 0, ex[g10])
        sc_tile(1, 1, 1, exf[g11])
        pv10b = pv_mm(1, 0, 1, ex[g10])
        pv_drain(1, 0, (pv10a, pv10b))
        sc_tile(1, 1, 2, exf[g11])
        oproj(0, nc.scalar, range(0, 2))
        sc_tile(1, 1, 3, exf[g11])
        oproj(0, nc.scalar, range(2, 4))
        sc_tile(1, 1, 4, exf[g11])
        sc_muls(1, 1, exf[g11], ex[g11])
        oproj(0, nc.scalar, range(4, 8))
        pv11a = pv_mm(1, 1, 0, ex[g11])
        pv11b = pv_mm(1, 1, 1, ex[g11])
        pv_drain(1, 1, (pv11a, pv11b))
        oproj(1, nc.scalar)

    return nc


_prog_cache = {}


def _get_program():
    if "nc" not in _prog_cache:
        _prog_cache["nc"] = build_program()
    return _prog_cache["nc"]


def _pt(a, nt):
    """[nt*128, l] -> [128, nt*l] partition-major contiguous."""
    l = a.shape[1]
    return np.ascontiguousarray(
        a.reshape(nt, P, l).transpose(1, 0, 2).reshape(P, nt * l))


def _prep_inputs(q, k, v, Wq, bq, Wk, bk, Wv, bv, Wo, bo, logits_bias,
                 key_padding_mask):
    """Build the 8 per-core input maps (host-side shard/compact/transpose)."""
    in_maps = []
    cast = lambda a: np.ascontiguousarray(a).astype(NP_CDT)
    per_batch = []
    for b in range(B):
        keep = np.nonzero(~np.asarray(key_padding_mask[b]))[0]
        nk = len(keep)
        assert nk <= LKP, f"unmasked key count {nk} exceeds LKP={LKP}"
        qb = cast(np.asarray(q[b]).T)                      # [D, LQ]
        qT0 = _pt(qb[:, 0:LQT], DT)
        qT1 = _pt(qb[:, LQT:LQ], DT)
        kc = cast(np.asarray(k[b])[keep].T)                # [D, nk]
        kT0 = np.zeros((D, KS0), NP_CDT)
        kT0[:, :min(nk, KS0)] = kc[:, :KS0]
        kT0 = _pt(kT0, DT)
        kT1 = np.zeros((D, LKP - KS0), NP_CDT)
        if nk > KS0:
            kT1[:, :nk - KS0] = kc[:, KS0:]
        kT1 = _pt(kT1, DT)
        vT = np.zeros((D, LKP), NP_CDT)
        vT[:, :nk] = cast(np.asarray(v[b])[keep].T)
        vT = _pt(vT, DT)                        # [128, (t, 640)]
        vT = np.ascontiguousarray(              # -> [128, (k, t, 128)]
            vT.reshape(P, DT, KT, P).transpose(0, 2, 1, 3).reshape(P, DT * LKP))
        ebT = np.zeros((LKP, LQ), NP_CDT)
        ebT[:nk] = np.exp(np.asarray(logits_bias[b])[:, keep]).T.astype(NP_CDT)
        ebT = _pt(ebT, KT)
        per_batch.append((qT0, qT1, kT0, kT1, vT, ebT))
    for g in range(GROUPS):
        sl = slice(M * g, M * (g + 1))
        wqT = _pt(cast(np.asarray(Wq)[sl, :].T), DT)
        wkT = _pt(cast(np.asarray(Wk)[sl, :].T), DT)
        wvT = _pt(cast(np.asarray(Wv)[sl, :].T), DT)
        woT = _pt(cast(np.asarray(Wo)[:, sl].T), M // P)
        bqg, bkg = np.asarray(bq)[sl], np.asarray(bk)[sl]
        bqk_t = np.stack([bqg[0:P], bkg[0:P], bqg[P:M], bkg[P:M]],
                         axis=1).astype(np.float32)
        bqk_t = np.ascontiguousarray(bqk_t)
        for b in range(B):
            qT0, qT1, kT0, kT1, vT, ebT = per_batch[b]
            in_maps.append({
                "qT0": qT0, "qT1": qT1, "kT0": kT0, "kT1": kT1, "vT": vT,
                "wqT": wqT, "wkT": wkT, "wvT": wvT, "woT": woT, "ebT": ebT,
                "bqk": bqk_t,
            })
    # core order: index = g * B + b  -> core for (b, g)
    return in_maps


def _combine(results, Wo, bv, bo):
    # (attn + bv) @ Wo.T + bo == attn @ Wo.T + (Wo @ bv + bo)
    bo_eff = (np.asarray(Wo, np.float32) @ np.asarray(bv, np.float32)
              + np.asarray(bo, np.float32))
    out = np.zeros((B, LQ, D), np.float32)
    for b in range(B):
        acc = np.zeros((D, LQ), np.float32)
        for g in range(GROUPS):
            acc += results[g * B + b]["outT"].astype(np.float32)
        out[b] = acc.T + bo_eff[None, :]
    return out


def kernel(**inputs):
    nc = _get_program()
    in_maps = _prep_inputs(**inputs)
    res = run_bass_kernel_spmd(nc, in_maps, core_ids=list(range(NCORES)))
    return _combine(res.results, inputs["Wo"], inputs["bv"], inputs["bo"])


# revision 19
# speedup vs baseline: 1.1878x; 1.1878x over previous
"""Bass/Trainium2 kernel for nn_BiasedCrossAttention (B=2, Lq=Lk=1024, D=1024, H=16).

Sharding: 8 cores = 2 batches x 4 head-groups (4 heads each, tensor-parallel).

Structure (v2 — rebuilt from trace analysis of the 81us baseline):
  - Host-side key compaction: masked keys are dropped exactly (softmax prob 0),
    K/V/bias gathered to the unmasked keys and padded to LKP=640 columns with
    bias=-30000 so pads contribute exactly nothing.
  - Inputs pre-tiled host-side to [128, t, l] contiguous; q/k split into
    first-use-sized chunks and DMA'd in consumption order so the first real
    matmul starts as soon as wq+q-half land (~13us) instead of after the
    full preamble.
  - PE warm/ramp dummies depend only on GpSimd-memset tiles (not on any DMA),
    so they fill the DMA preamble instead of serializing after it.
  - Softmax: scores stay in PSUM; GpSimd drains them with a fused
    (s*0.125 + bias) scalar_tensor_tensor into f32 SBUF (psum banks recycle at
    GpSimd speed, and the eb-dependence lives on the GpSimd queue only, so
    PE-side projection drains on DVE can never head-of-line block on eb);
    ScalarE then does a single Exp per tile straight to bf16.  bias is shipped
    raw in fp16 (no host-side exp), masked/padded keys get -30000.
  - Softmax denominator via the ones-columns trick in vh (free: PV matmul
    output partitions 64..127 replicate the denominator), reciprocal via the
    single-op DVE reciprocal_approx_fast instead of a ScalarE Ln/Exp chain.
  - bv and bo are folded host-side into a single output bias
    (out += (attn + bv) @ Wo.T + bo == attn @ Wo.T + (Wo@bv + bo)), dropping
    the vproj ones-row (VDT 9 -> 8).
  - o_proj psum->bf16 casts split GpSimd (lt=0) / DVE (lt=1) so the final
    cast+DMA tail is not queued behind other work.

Per core (batch b, head-group g):
  - qhT/khT [m=256, L] projections (m = group's head dims), bias folded into
    the psum->sbuf DVE copy
  - vh [LKP, per-head 64 v-cols + 64 ones-cols]
  - probs = exp(scores/8 + bias), no max-subtraction (safe at these magnitudes)
  - o_proj partial outT [D, Lq] in bf16, summed over the 4 groups host-side.
"""

import numpy as np
import ml_dtypes

import bass_rust
import concourse.bass as bass
import concourse.tile as tile
import concourse.mybir as mybir
from concourse.bass_utils import run_bass_kernel_spmd
from concourse.vector_clock import ScopedClock

F32 = mybir.dt.float32
F16 = mybir.dt.float16
BF16 = mybir.dt.bfloat16
AT = mybir.ActivationFunctionType
ALU = mybir.AluOpType

B, LQ, LK, D, H = 2, 1024, 1024, 1024, 16
DH = D // H              # 64
GROUPS = 4               # head-groups across cores (x B batches = 8 cores)
GH = H // GROUPS         # heads per group
M = GH * DH              # 256 projected dims per group
NCORES = 8
P = 128
DT = D // P              # 8 d-tiles
LKP = 640                # padded compacted key count
KT = LKP // P            # 5 lk-tiles
KS0 = 512                # first k-chunk (matches kproj_a)
LQT = 512                # lq tile (psum free limit)
NLT = LQ // LQT          # 2 lq-tiles
SCALE = 1.0 / np.sqrt(DH)
NEGB = -30000.0          # bias for masked/padded keys: exp underflows to 0
NDUMMY = 16              # PE warm/ramp matmuls covering the DMA preamble
DUMW = 384               # dummy matmul free size

CDT = BF16               # matmul compute dtype
NP_CDT = ml_dtypes.bfloat16

_counter = [0]


def _split_waits_in_list(nc, insts):
    """This walrus build rejects >1 embedded sync-wait per instruction; move
    extra waits onto standalone EventSemaphore instructions just before."""
    out = []
    for ins in insts:
        si = getattr(ins, "sync_info", None)
        if si is not None and len(si.on_wait) > 1:
            extra = list(si.on_wait[:-1])
            del si.on_wait[:-1]
            for w in extra:
                _counter[0] += 1
                ev = bass_rust.InstEventSemaphore(
                    name=f"I-xw{_counter[0]}", ins=[], outs=[])
                ev.engine = ins.engine
                ev.sync_info = mybir.SyncInfo(on_wait=[w], on_update=[])
                try:
                    ev.debug = ins.debug
                except Exception:
                    pass
                nc.register_instruction(ev)
                out.append(ev)
        out.append(ins)
    return out


class PatchedTileContext(tile.TileContext):
    def _lower_ordered_insts(self, ordered):
        for name in list(ordered.keys()):
            ordered[name] = _split_waits_in_list(self.nc, ordered[name])
        return super()._lower_ordered_insts(ordered)

    def _drain_and_barrier(self, tick_clock, wait_clock):
        nc = self.nc
        drain_inst = nc.sync.drain()
        wait_clock.add_sem_waits(
            drain_inst.ins, ScopedClock({None: tick_clock.global_clock}))
        si = drain_inst.ins.sync_info
        waits = list(si.on_wait)
        if len(waits) > 1:
            del si.on_wait[1:]
            for w in waits[1:]:
                nop = nc.sync.nop(nofuse=True)
                if nop.ins.sync_info is None:
                    nop.ins.sync_info = mybir.SyncInfo(on_wait=[], on_update=[])
                nop.ins.sync_info.on_wait.append(w)
        nc.all_engine_barrier()
        assert self.sems is not None
        popped = nc._tile_sem_poison_stack.pop()
        assert popped is self._sem_poison
        nc.clear_and_free_semaphores(list(self.sems.allocated().values()))
        nc.all_engine_barrier()


def build_program():
    nc = bass.Bass()

    HE = 2 * DH

    # inputs are host-pre-tiled to [128, n*l] contiguous (partition-major)
    qT0 = nc.dram_tensor("qT0", [P, DT * LQT], CDT, kind="ExternalInput")
    qT1 = nc.dram_tensor("qT1", [P, DT * LQT], CDT, kind="ExternalInput")
    kT0 = nc.dram_tensor("kT0", [P, DT * KS0], CDT, kind="ExternalInput")
    kT1 = nc.dram_tensor("kT1", [P, DT * (LKP - KS0)], CDT, kind="ExternalInput")
    vT = nc.dram_tensor("vT", [P, DT * LKP], CDT, kind="ExternalInput")
    wqT = nc.dram_tensor("wqT", [P, DT * M], CDT, kind="ExternalInput")
    wkT = nc.dram_tensor("wkT", [P, DT * M], CDT, kind="ExternalInput")
    wvT = nc.dram_tensor("wvT", [P, DT * M], CDT, kind="ExternalInput")
    woT = nc.dram_tensor("woT", [P, (M // P) * D], CDT, kind="ExternalInput")
    ebT = nc.dram_tensor("ebT", [P, KT * LQ], CDT, kind="ExternalInput")
    bqk = nc.dram_tensor("bqk", [P, 4], F32, kind="ExternalInput")
    outT = nc.dram_tensor("outT", [D, LQ], CDT, kind="ExternalOutput")

    from contextlib import ExitStack
    with PatchedTileContext(nc) as tc, ExitStack() as ctx:
        consts = ctx.enter_context(tc.tile_pool(name="consts", bufs=1))
        tmp = ctx.enter_context(tc.tile_pool(name="tmp", bufs=10))
        rcp = ctx.enter_context(tc.tile_pool(name="rcp", bufs=6))
        exps = ctx.enter_context(tc.tile_pool(name="exps", bufs=18))
        outp = ctx.enter_context(tc.tile_pool(name="outp", bufs=6))
        ps_pp = ctx.enter_context(tc.tile_pool(name="ps_pp", bufs=2, space="PSUM"))
        ps_sc = ctx.enter_context(tc.tile_pool(name="ps_sc", bufs=2, space="PSUM"))
        ps_pv = ctx.enter_context(tc.tile_pool(name="ps_pv", bufs=2, space="PSUM"))

        def flat(t3):
            return t3.rearrange("p t l -> p (t l)")

        # ---- input loads: consumption-ordered, contiguous ----------------
        # v is shipped k-tile-major so vproj(k) can start per 0.26MB chunk.
        bq_t = consts.tile([P, 4], F32, name="bq_t")
        nc.sync.dma_start(bq_t[:], bqk[:])
        wq_a = consts.tile([P, DT, M], CDT, name="wq_a")
        nc.sync.dma_start(flat(wq_a[:]), wqT[:])
        q_a = [consts.tile([P, DT, LQT], CDT, name=f"q_a{t}") for t in range(2)]
        nc.sync.dma_start(flat(q_a[0][:]), qT0[:])
        wk_a = consts.tile([P, DT, M], CDT, name="wk_a")
        nc.sync.dma_start(flat(wk_a[:]), wkT[:])
        k_a0 = consts.tile([P, DT, KS0], CDT, name="k_a0")
        nc.sync.dma_start(flat(k_a0[:]), kT0[:])
        eb_a = [consts.tile([P, LQ], CDT, name=f"eb_a{k}") for k in range(KT)]
        nc.sync.dma_start(eb_a[0][:], ebT[:, 0 * LQ:1 * LQ])
        nc.sync.dma_start(flat(q_a[1][:]), qT1[:])
        nc.sync.dma_start(eb_a[1][:], ebT[:, 1 * LQ:2 * LQ])
        k_a1 = consts.tile([P, DT, LKP - KS0], CDT, name="k_a1")
        nc.sync.dma_start(flat(k_a1[:]), kT1[:])
        wv_a = consts.tile([P, DT, M], CDT, name="wv_a")
        nc.sync.dma_start(flat(wv_a[:]), wvT[:])
        v_a = consts.tile([P, KT, DT, P], CDT, name="v_a")
        vf = v_a[:].rearrange("p k t c -> p (k t c)")
        KCH = DT * P
        nc.sync.dma_start(vf[:, 0 * KCH:1 * KCH], vT[:, 0 * KCH:1 * KCH])
        nc.sync.dma_start(vf[:, 1 * KCH:2 * KCH], vT[:, 1 * KCH:2 * KCH])
        nc.sync.dma_start(eb_a[2][:], ebT[:, 2 * LQ:3 * LQ])
        nc.sync.dma_start(vf[:, 2 * KCH:3 * KCH], vT[:, 2 * KCH:3 * KCH])
        nc.sync.dma_start(eb_a[3][:], ebT[:, 3 * LQ:4 * LQ])
        nc.sync.dma_start(vf[:, 3 * KCH:4 * KCH], vT[:, 3 * KCH:4 * KCH])
        nc.sync.dma_start(eb_a[4][:], ebT[:, 4 * LQ:5 * LQ])
        nc.sync.dma_start(vf[:, 4 * KCH:5 * KCH], vT[:, 4 * KCH:5 * KCH])
        wo_a = consts.tile([P, M // P, D], CDT, name="wo_a")
        nc.sync.dma_start(flat(wo_a[:]), woT[:])

        # ---- warmers ------------------------------------------------------
        # ones/vh memsets on GpSimd, exp table load on Scalar, then a run of
        # DMA-independent dummy matmuls that keep the PE busy (and its p-state
        # ramped) through the DMA preamble.
        ones0 = consts.tile([P, P], CDT, name="ones0")
        nc.gpsimd.memset(ones0[:], 1.0)
        ones1 = consts.tile([P, DUMW], CDT, name="ones1")
        nc.gpsimd.memset(ones1[:], 1.0)
        vh = [consts.tile([P, GH * HE], CDT, name=f"vh{k}") for k in range(KT)]
        for k in range(KT):
            nc.gpsimd.memset(vh[k][:], 1.0)
        wsrc = consts.tile([P, 8], F32, name="wsrc")
        nc.gpsimd.memset(wsrc[:], 0.0)
        wdst = tmp.tile([P, 8], F32, name="wdst")
        nc.scalar.activation(wdst[:], wsrc[:], AT.Exp)
        for w in range(NDUMMY):
            dmy = ps_pp.tile([P, DUMW], F32, name="dmy", tag="pp")
            nc.tensor.matmul(dmy[:], ones0[:], ones1[:], start=True, stop=True)

        # ---- projections --------------------------------------------------
        qh = [consts.tile([P, LQ], CDT, name=f"qh{p}") for p in range(M // P)]
        kh = [consts.tile([P, LKP], CDT, name=f"kh{p}") for p in range(M // P)]

        def qproj(lt):
            for p in range(M // P):
                pq = ps_pp.tile([P, LQT], F32, name="pq", tag="pp")
                for i in range(DT):
                    nc.tensor.matmul(pq[:], wq_a[:, i, P * p:P * (p + 1)],
                                     q_a[lt][:, i, :],
                                     start=(i == 0), stop=(i == DT - 1))
                nc.vector.tensor_scalar_add(qh[p][:, LQT * lt:LQT * (lt + 1)],
                                            pq[:], bq_t[:, 2 * p:2 * p + 1])

        def kproj(src, c0, c1):
            for p in range(M // P):
                pk = ps_pp.tile([P, c1 - c0], F32, name="pk", tag="pp")
                for i in range(DT):
                    nc.tensor.matmul(pk[:], wk_a[:, i, P * p:P * (p + 1)],
                                     src[:, i, :],
                                     start=(i == 0), stop=(i == DT - 1))
                nc.vector.tensor_scalar_add(kh[p][:, c0:c1], pk[:],
                                            bq_t[:, 2 * p + 1:2 * p + 2])

        def vproj(k):
            # vh [LKP, GH*128]: per head 64 v-cols + 64 ones-cols; the PV
            # matmul then replicates the denominator across partitions 64..127
            pv = ps_pp.tile([P, M], F32, name="pvproj", tag="pp")
            for i in range(DT):
                nc.tensor.matmul(pv[:], v_a[:, k, i, :], wv_a[:, i, :],
                                 start=(i == 0), stop=(i == DT - 1))
            nc.vector.tensor_copy(
                vh[k][:, :].rearrange("p (h e) -> p h e", e=HE)[:, :, 0:DH],
                pv[:].rearrange("p (h e) -> p h e", e=DH))

        # ---- attention + output projection, lq-tile major -----------------
        attnT = [consts.tile([P, LQ], CDT, name=f"attnT{p}") for p in range(M // P)]

        def sc_tile(lt, hp, k, exf_t):
            # scores for one k-tile -> psum [128,(hl,512)]; ScalarE drains it
            # with a single Exp (psum recycle is eb-independent), GpSimd then
            # multiplies in exp(bias) (SBUF-only; GpSimd cannot touch PSUM).
            lq = slice(LQT * lt, LQT * (lt + 1))
            sps = ps_sc.tile([P, LQ], F32, name="sps", tag="sc")
            for hl in range(2):
                nc.tensor.matmul(sps[:, LQT * hl:LQT * (hl + 1)],
                                 kh[hp][DH * hl:DH * (hl + 1), P * k:P * (k + 1)],
                                 qh[hp][DH * hl:DH * (hl + 1), lq],
                                 start=True, stop=True)
            exf = tmp.tile([P, LQ], CDT, name="exf")
            nc.scalar.activation(exf[:], sps[:], AT.Exp, scale=float(SCALE))
            exf_t.append(exf)

        def sc_muls(lt, hp, exf_t, ex_t, kr=None):
            # one DVE op per k-tile: eb broadcasts over the two hl halves via
            # a stride-0 AP dim (GpSimd is ~2.8x slower per op - keep it out)
            lq = slice(LQT * lt, LQT * (lt + 1))
            for k in (range(KT) if kr is None else kr):
                ex = exps.tile([P, LQ], CDT, name="ex")
                ebs3 = eb_a[k][:, lq].rearrange("p (o l) -> p o l", o=1)
                exf3 = exf_t[k][:].rearrange("p (h l) -> p h l", h=2)
                exf3b, ebs3b = bass.broadcast_tensor_aps(exf3, ebs3)
                nc.vector.tensor_mul(ex[:].rearrange("p (h l) -> p h l", h=2),
                                     exf3b, ebs3b)
                ex_t.append(ex)

        def pv_mm(lt, hp, hl, ex_t):
            h = 2 * hp + hl
            pvo = ps_pv.tile([P, LQT], F32, name="pvo", tag="pv")
            for k in range(KT):
                nc.tensor.matmul(pvo[:], vh[k][:, HE * h:HE * (h + 1)],
                                 ex_t[k][:, LQT * hl:LQT * (hl + 1)],
                                 start=(k == 0), stop=(k == KT - 1))
            return pvo

        def pv_drain(lt, hp, pvos):
            # attn = pv * (1/denominator); denominator replicated in 64:128.
            # 1/den = exp(-ln(den)) on ScalarE straight from psum (Ln+Exp
            # share one activation-table set; DVE reciprocal is ~6 cyc/elem
            # and the custom-DVE fast recip is rejected by this walrus build).
            lq = slice(LQT * lt, LQT * (lt + 1))
            for hl in range(2):
                lnd = rcp.tile([DH, LQT], F32, name="lnd")
                nc.scalar.activation(lnd[:], pvos[hl][DH:P, :], AT.Ln)
                rec = rcp.tile([DH, LQT], F32, name="rec")
                nc.scalar.activation(rec[:], lnd[:], AT.Exp, scale=-1.0)
                nc.vector.tensor_mul(attnT[hp][DH * hl:DH * (hl + 1), lq],
                                     pvos[hl][0:DH, :], rec[:])

        def oproj(lt, cast_eng, ots=None):
            lq = slice(LQT * lt, LQT * (lt + 1))
            for ot in (range(DT) if ots is None else ots):
                po = ps_pp.tile([P, LQT], F32, name="po", tag="pp")
                for p in range(M // P):
                    nc.tensor.matmul(po[:], wo_a[:, p, P * ot:P * (ot + 1)],
                                     attnT[p][:, lq],
                                     start=(p == 0), stop=(p == M // P - 1))
                osb = outp.tile([P, LQT], CDT, name="osb")
                if cast_eng is nc.scalar:
                    nc.scalar.copy(osb[:], po[:])
                else:
                    cast_eng.tensor_copy(osb[:], po[:])
                nc.sync.dma_start(outT[P * ot:P * (ot + 1), lq], osb[:])

        # ---- schedule -----------------------------------------------------
        # Two pacing facts drive this order: (1) the scores psum pool recycles
        # at ScalarE exp speed (~1us/tile vs ~0.45us PE/tile), so score tiles
        # are interleaved with dense fill (projections, vproj chunks, pv
        # blocks, oproj chunks); (2) the PE drops to its mid p-state after
        # idle/sparse stretches, so the fill also keeps the clock ramped.
        # pv drains are emitted right after their pv pair so the 2-slot pv
        # psum pool unblocks quickly.
        exf = [[] for _ in range(4)]   # raw exp(scores/8) per (lt,hp)
        ex = [[] for _ in range(4)]    # exp * exp(bias), PV input
        g00, g01, g10, g11 = 0, 1, 2, 3

        qproj(0)
        kproj(k_a0, 0, KS0)
        sc_tile(0, 0, 0, exf[g00])
        sc_tile(0, 0, 1, exf[g00])
        qproj(1)
        sc_tile(0, 0, 2, exf[g00])
        sc_tile(0, 0, 3, exf[g00])
        kproj(k_a1, KS0, LKP)
        sc_tile(0, 1, 0, exf[g01])
        sc_tile(0, 1, 1, exf[g01])
        vproj(0)
        sc_tile(0, 1, 2, exf[g01])
        vproj(1)
        sc_tile(0, 1, 3, exf[g01])
        vproj(2)
        sc_tile(0, 0, 4, exf[g00])
        sc_muls(0, 0, exf[g00], ex[g00])
        vproj(3)
        sc_tile(0, 1, 4, exf[g01])
        sc_muls(0, 1, exf[g01], ex[g01])
        vproj(4)
        sc_tile(1, 0, 0, exf[g10])
        sc_tile(1, 0, 1, exf[g10])
        pv00a = pv_mm(0, 0, 0, ex[g00])
        sc_tile(1, 0, 2, exf[g10])
        pv00b = pv_mm(0, 0, 1, ex[g00])
        pv_drain(0, 0, (pv00a, pv00b))
        sc_tile(1, 0, 3, exf[g10])
        pv01a = pv_mm(0, 1, 0, ex[g01])
        sc_tile(1, 0, 4, exf[g10])
        sc_muls(1, 0, exf[g10], ex[g10])
        pv01b = pv_mm(0, 1, 1, ex[g01])
        pv_drain(0, 1, (pv01a, pv01b))
        sc_tile(1, 1, 0, exf[g11])
        pv10a = pv_mm(1, 0, 0, ex[g10])
        sc_tile(1, 1, 1, exf[g11])
        pv10b = pv_mm(1, 0, 1, ex[g10])
        pv_drain(1, 0, (pv10a, pv10b))
        sc_tile(1, 1, 2, exf[g11])
        oproj(0, nc.vector, range(0, 2))
        sc_tile(1, 1, 3, exf[g11])
        oproj(0, nc.vector, range(2, 4))
        sc_tile(1, 1, 4, exf[g11])
        sc_muls(1, 1, exf[g11], ex[g11])
        oproj(0, nc.vector, range(4, 8))
        pv11a = pv_mm(1, 1, 0, ex[g11])
        pv11b = pv_mm(1, 1, 1, ex[g11])
        pv_drain(1, 1, (pv11a, pv11b))
        oproj(1, nc.scalar)

    return nc


_prog_cache = {}


def _get_program():
    if "nc" not in _prog_cache:
        _prog_cache["nc"] = build_program()
    return _prog_cache["nc"]


def _pt(a, nt):
    """[nt*128, l] -> [128, nt*l] partition-major contiguous."""
    l = a.shape[1]
    return np.ascontiguousarray(
        a.reshape(nt, P, l).transpose(1, 0, 2).reshape(P, nt * l))


def _prep_inputs(q, k, v, Wq, bq, Wk, bk, Wv, bv, Wo, bo, logits_bias,
                 key_padding_mask):
    """Build the 8 per-core input maps (host-side shard/compact/transpose)."""
    in_maps = []
    cast = lambda a: np.ascontiguousarray(a).astype(NP_CDT)
    per_batch = []
    for b in range(B):
        keep = np.nonzero(~np.asarray(key_padding_mask[b]))[0]
        nk = len(keep)
        assert nk <= LKP, f"unmasked key count {nk} exceeds LKP={LKP}"
        qb = cast(np.asarray(q[b]).T)                      # [D, LQ]
        qT0 = _pt(qb[:, 0:LQT], DT)
        qT1 = _pt(qb[:, LQT:LQ], DT)
        kc = cast(np.asarray(k[b])[keep].T)                # [D, nk]
        kT0 = np.zeros((D, KS0), NP_CDT)
        kT0[:, :min(nk, KS0)] = kc[:, :KS0]
        kT0 = _pt(kT0, DT)
        kT1 = np.zeros((D, LKP - KS0), NP_CDT)
        if nk > KS0:
            kT1[:, :nk - KS0] = kc[:, KS0:]
        kT1 = _pt(kT1, DT)
        vT = np.zeros((D, LKP), NP_CDT)
        vT[:, :nk] = cast(np.asarray(v[b])[keep].T)
        vT = _pt(vT, DT)                        # [128, (t, 640)]
        vT = np.ascontiguousarray(              # -> [128, (k, t, 128)]
            vT.reshape(P, DT, KT, P).transpose(0, 2, 1, 3).reshape(P, DT * LKP))
        ebT = np.zeros((LKP, LQ), NP_CDT)
        ebT[:nk] = np.exp(np.asarray(logits_bias[b])[:, keep]).T.astype(NP_CDT)
        ebT = _pt(ebT, KT)
        per_batch.append((qT0, qT1, kT0, kT1, vT, ebT))
    for g in range(GROUPS):
        sl = slice(M * g, M * (g + 1))
        wqT = _pt(cast(np.asarray(Wq)[sl, :].T), DT)
        wkT = _pt(cast(np.asarray(Wk)[sl, :].T), DT)
        wvT = _pt(cast(np.asarray(Wv)[sl, :].T), DT)
        woT = _pt(cast(np.asarray(Wo)[:, sl].T), M // P)
        bqg, bkg = np.asarray(bq)[sl], np.asarray(bk)[sl]
        bqk_t = np.stack([bqg[0:P], bkg[0:P], bqg[P:M], bkg[P:M]],
                         axis=1).astype(np.float32)
        bqk_t = np.ascontiguousarray(bqk_t)
        for b in range(B):
            qT0, qT1, kT0, kT1, vT, ebT = per_batch[b]
            in_maps.append({
                "qT0": qT0, "qT1": qT1, "kT0": kT0, "kT1": kT1, "vT": vT,
                "wqT": wqT, "wkT": wkT, "wvT": wvT, "woT": woT, "ebT": ebT,
                "bqk": bqk_t,
            })
    # core order: index = g * B + b  -> core for (b, g)
    return in_maps


def _combine(results, Wo, bv, bo):
    # (attn + bv) @ Wo.T + bo == attn @ Wo.T + (Wo @ bv + bo)
    bo_eff = (np.asarray(Wo, np.float32) @ np.asarray(bv, np.float32)
              + np.asarray(bo, np.float32))
    out = np.zeros((B, LQ, D), np.float32)
    for b in range(B):
        acc = np.zeros((D, LQ), np.float32)
        for g in range(GROUPS):
            acc += results[g * B + b]["outT"].astype(np.float32)
        out[b] = acc.T + bo_eff[None, :]
    return out


def kernel(**inputs):
    nc = _get_program()
    in_maps = _prep_inputs(**inputs)
    res = run_bass_kernel_spmd(nc, in_maps, core_ids=list(range(NCORES)))
    return _combine(res.results, inputs["Wo"], inputs["bv"], inputs["bo"])
